# revision 29
# baseline (speedup 1.0000x reference)
"""TRN2 Bass kernel for nn_BalancedHamiltonLayer.

Math: out[n,k,j] = sum_{r,s,i} x[n,s,i] * factors_B[r,j,i] * H(A)[r,k,s] + bias
collapses to a single dense matmul  out = x2d @ W + bias  with
W[(s,i),(k,j)] = sum_r H[r,k,s] * B[r,j,i]  (a 1024x1024 matrix folded on host
in float64).

Sharding: data-parallel over the 8192 token rows across 8 NeuronCores
(1024 rows each); W replicated.  The matmul runs in fp16 on the PE
(full-rate, FWL weight loads; fp32 PSUM accumulation).  Outputs are
stored fp16 (rounding ~5e-4 rel, tolerance is 2e-2) halving store
traffic; bias is added on the host during the gather.

Layouts are partition-major in DRAM so every DMA descriptor is a 2-8KB
contiguous run:
  xT[p, m, k, t] = x[m*128+t, k*128+p]   (lhsT tiles slice out [p, t])
  w [p, k, j]    = W[k*128+p, j]         (rhs slices [p, j-half])

Schedule: the single HWDGE ring drains sequentially and a chunk is
usable only at its 16th engine-completion, so chunk-ready time is
~(3.3us + 4-4.7us per cumulative MB ahead of it).  Front chunks are
split into 128KB halves (x m1-3 by k-half, W k1/k2 by n-half) and
ordered so every chunk's ready-time minus its consumption offset is
balanced (~the theoretical floor base + slope^2/7).  Nine cold warmup
matmuls on a zeroed tile keep the PE busy from the post-barrier point
to the DMA-gated stream start so HAM unthrottles to 2.4 GHz before the
real matmuls.  Phase 1 holds m0..3 in all 8 PSUM banks, k advancing in
DMA-arrival order (PE FIFO = emission order); its k7 pass runs m0
first so the copies free banks before phase 2 claims them.  Phase 2 is
n-major per m-tile so each half-bank closes 8 matmuls early and its
copy+store overlaps the other half; the last half is two 256-col
quarters so the tail after the final matmul is a [128,256] copy + 64KB
store.
"""

import numpy as np
import concourse.bacc as bacc
import concourse.mybir as mybir
import concourse.tile as tile
from concourse.bass_utils import run_bass_kernel_spmd

B, T, D = 4, 2048, 1024
RANK, FACTOR, SUB = 8, 64, 4
S = 4 * SUB  # 16
NCORES = 8
NTOK = B * T // NCORES  # 1024 token rows per core
P = 128
KT = D // P     # 8 contraction chunks
MT = NTOK // P  # 8 token tiles per core
NH = 512        # f_out half (one PSUM bank)

_cached_nc = None


def build_module():
    global _cached_nc
    if _cached_nc is not None:
        return _cached_nc
    nc = bacc.Bacc("TRN2", target_bir_lowering=False, debug=False)
    xT = nc.dram_tensor("xT", [P, MT, KT, P], mybir.dt.float16, kind="ExternalInput").ap()
    w = nc.dram_tensor("w", [P, KT, D], mybir.dt.float16, kind="ExternalInput").ap()
    out = nc.dram_tensor("out", [NTOK, D], mybir.dt.float16, kind="ExternalOutput").ap()

    with tile.TileContext(nc) as tc:
        with (
            tc.tile_pool(name="wp", bufs=1) as wp,
            tc.tile_pool(name="xp", bufs=1) as xp,
            tc.tile_pool(name="op", bufs=4) as op,
            tc.tile_pool(name="ps", bufs=4, space="PSUM") as ps,
        ):
            # PE HAM pre-warm: a couple of matmuls on a zeroed SBUF tile
            # start the activity window while the first loads are in
            # flight.  Tuned to end right as the first real operands
            # land — more would block the PE FIFO behind junk work.
            g = xp.tile([P, NH], mybir.dt.float16, tag="warm", name="g")
            nc.gpsimd.memset(g[:], 0.0)

            # Singles (256KB each, 2KB/partition contiguous) so every
            # ~0.95us another chunk unlocks matmuls; phase-2 x as pairs.
            KH = KT // 2
            # m0..3 split into k-halves so the phase-1-critical first
            # half needs fewer bytes ahead of it on the ring.
            xmh = {
                (m, h): xp.tile([P, 1, KH, P], mybir.dt.float16, tag=f"x{m}{h}", name=f"xm{m}{h}")
                for m in (0, 1, 2, 3) for h in (0, 1)
            }
            xm45 = xp.tile([P, 2, KT, P], mybir.dt.float16, tag="x45", name="xm45")
            xm67 = xp.tile([P, 2, KT, P], mybir.dt.float16, tag="x67", name="xm67")
            WSPLIT = (1, 2)  # W chunks loaded as n-halves
            wk = {
                k: wp.tile([P, 1, D], mybir.dt.float16, tag=f"w{k}", name=f"wk{k}")
                for k in range(KT) if k not in WSPLIT
            }
            wkh = {
                (k, n): wp.tile([P, 1, NH], mybir.dt.float16, tag=f"w{k}{n}", name=f"wk{k}{n}")
                for k in WSPLIT for n in (0, 1)
            }

            def xs(m, k):
                # lhsT [128 contraction rows, 128 tokens] for tile (m, k)
                if m < 4:
                    return xmh[(m, k // KH)][:, 0, k % KH, :]
                pair = xm45 if m < 6 else xm67
                return pair[:, m % 2, k, :]

            def wr(k, n):
                # rhs [128 contraction rows, 512 outs]
                if k in WSPLIT:
                    return wkh[(k, n)][:, 0, :]
                return wk[k][:, 0, n * NH:(n + 1) * NH]

            def wrc(k, c0, cw):
                # rhs [128 contraction rows, cw outs] at column offset c0
                if k in WSPLIT:
                    return wkh[(k, c0 // NH)][:, 0, c0 % NH:c0 % NH + cw]
                return wk[k][:, 0, c0:c0 + cw]

            # One HWDGE ring (sync), deadline order: the ring drains
            # sequentially, and the completion skew of the slowest SDMA
            # engine grows with cumulative bytes, so every chunk's
            # ready-time is ~(base + slope * bytes-ahead-of-it).  The
            # 128KB halves keep the early phase-1 dependencies low on
            # that line; the second halves ride after wk2.
            loads = [
                (wk[0][:], w[:, 0:1, :]),
                (xmh[(0, 0)][:], xT[:, 0:1, 0:KH]),
                (xmh[(1, 0)][:], xT[:, 1:2, 0:KH]),
                (wkh[(1, 0)][:], w[:, 1:2, 0:NH]),
                (xmh[(2, 0)][:], xT[:, 2:3, 0:KH]),
                (xmh[(3, 0)][:], xT[:, 3:4, 0:KH]),
                (wkh[(1, 1)][:], w[:, 1:2, NH:D]),
                (wkh[(2, 0)][:], w[:, 2:3, 0:NH]),
                (wkh[(2, 1)][:], w[:, 2:3, NH:D]),
                (wk[3][:], w[:, 3:4, :]),
                (wk[4][:], w[:, 4:5, :]),
                (xmh[(0, 1)][:], xT[:, 0:1, KH:KT]),
                (xmh[(1, 1)][:], xT[:, 1:2, KH:KT]),
                (xmh[(2, 1)][:], xT[:, 2:3, KH:KT]),
                (xmh[(3, 1)][:], xT[:, 3:4, KH:KT]),
                (wk[5][:], w[:, 5:6, :]),
                (wk[6][:], w[:, 6:7, :]),
                (wk[7][:], w[:, 7:8, :]),
                (xm45[:], xT[:, 4:6]),
                (xm67[:], xT[:, 6:8]),
            ]
            for da, sa in loads:
                nc.sync.dma_start(da, sa)

            with nc.named_scope("mm"):
                pts = {
                    m: {
                        n: ps.tile([P, NH], mybir.dt.float32, tag=f"ps{n}", name=f"pt{m}_{n}")
                        for n in range(2)
                    }
                    for m in range(4)
                }
                # Cold warmups x 427ns bridge the PE from the post-
                # barrier point to the DMA-gated stream start (~5.3us in)
                # with no idle window, so HAM fires before the real
                # matmuls and every one of them runs at 2.4 GHz.
                for i in range(9):
                    nc.tensor.matmul(
                        pts[0][0][:], g[:, :P], g[:], start=True, stop=True
                    )

                # Phase 1: emission order tracks DMA arrival.
                def mm(m, k):
                    for n in range(2):
                        nc.tensor.matmul(
                            pts[m][n][:], xs(m, k), wr(k, n),
                            start=(k == 0), stop=(k == KT - 1),
                        )

                def mm1(m, k, n):
                    nc.tensor.matmul(
                        pts[m][n][:], xs(m, k), wr(k, n),
                        start=(k == 0), stop=(k == KT - 1),
                    )

                mm(0, 0); mm(1, 0)
                mm1(0, 1, 0); mm1(1, 1, 0)
                mm(2, 0); mm(3, 0)
                mm1(0, 1, 1); mm1(1, 1, 1)
                mm(2, 1); mm(3, 1)
                for m in range(4):
                    mm1(m, 2, 0)
                for m in range(4):
                    mm1(m, 2, 1)
                for k in range(3, KT):
                    for m in range(4):
                        mm(m, k)

                def emit_full(m, pt):
                    o = op.tile([P, D], mybir.dt.float16, tag="o", name="o")
                    for n in range(2):
                        nc.vector.tensor_copy(o[:, n * NH:(n + 1) * NH], pt[n][:])
                    nc.sync.dma_start(out[m * P:(m + 1) * P, :], o[:])

                for m in range(4):
                    emit_full(m, pts[m])

                # Phase 2: n-major per m so each half closes early.  The
                # very last half is split into two 256-col quarters so
                # the final copy+store tail after the last matmul is a
                # [128,256] copy + 64KB store instead of twice that.
                for m in range(4, MT):
                    last = m == MT - 1
                    pieces = [(0, NH), (NH, NH)] if not last else [
                        (0, NH), (NH, NH // 2), (NH + NH // 2, NH // 2)
                    ]
                    for n, (c0, cw) in enumerate(pieces):
                        pt = ps.tile([P, cw], mybir.dt.float32, tag=f"ps{min(n,1)}", name=f"p2_{m}_{n}")
                        for k in range(KT):
                            nc.tensor.matmul(
                                pt[:], xs(m, k), wrc(k, c0, cw),
                                start=(k == 0), stop=(k == KT - 1),
                            )
                        o = op.tile([P, cw], mybir.dt.float16, tag="oh", name="oh")
                        nc.vector.tensor_copy(o[:], pt[:])
                        nc.sync.dma_start(
                            out[m * P:(m + 1) * P, c0:c0 + cw], o[:]
                        )
    nc.compile()
    _cached_nc = nc
    return nc


def _construct_hamilton(A):
    # A: [rank, 4, sub, sub] -> [rank, 4*sub, 4*sub]
    r, i, j, k = A[:, 0], A[:, 1], A[:, 2], A[:, 3]
    return np.concatenate(
        [
            np.concatenate([r, -i, -j, -k], axis=2),
            np.concatenate([i, r, -k, j], axis=2),
            np.concatenate([j, k, r, -i], axis=2),
            np.concatenate([k, -j, i, r], axis=2),
        ],
        axis=1,
    )


def build_in_maps(x, A, factors_B):
    H = _construct_hamilton(np.asarray(A, dtype=np.float64))  # [r, k, s]
    Bf = np.asarray(factors_B, dtype=np.float64)  # [r, j, i]
    # W[(s,i),(k,j)] = sum_r H[r,k,s] * B[r,j,i]
    W = np.einsum("rks,rji->sikj", H, Bf).reshape(D, D).astype(np.float16)
    # w[p, k, j] = W[k*128+p, j] -> per-partition 16KB contiguous
    whost = np.ascontiguousarray(W.reshape(KT, P, D).transpose(1, 0, 2))

    x2 = np.asarray(x, dtype=np.float16).reshape(NCORES, NTOK, D)
    in_maps = []
    for c in range(NCORES):
        # xT[p, m, k, t] = x_core[m*128+t, k*128+p]
        xs = np.ascontiguousarray(
            x2[c].reshape(MT, P, KT, P).transpose(3, 0, 2, 1)
        )
        in_maps.append({"xT": xs, "w": whost})
    return in_maps


def kernel(x, A, factors_B, bias):
    nc = build_module()
    in_maps = build_in_maps(x, A, factors_B)
    br = run_bass_kernel_spmd(nc, in_maps, core_ids=list(range(NCORES)))
    out = np.concatenate([r["out"] for r in br.results], axis=0)
    out = out.astype(np.float32) + np.asarray(bias, dtype=np.float32)[None, :]
    return out.reshape(B, T, D).astype(np.float32)


# revision 30
# speedup vs baseline: 1.0689x; 1.0689x over previous
"""TRN2 Bass kernel for nn_BalancedHamiltonLayer.

Math: out[n,k,j] = sum_{r,s,i} x[n,s,i] * factors_B[r,j,i] * H(A)[r,k,s] + bias
collapses to a single dense matmul  out = x2d @ W + bias  with
W[(s,i),(k,j)] = sum_r H[r,k,s] * B[r,j,i]  (a 1024x1024 matrix folded on host
in float64).

Sharding: data-parallel over the 8192 token rows across 8 NeuronCores
(1024 rows each); W replicated.  The matmul runs in fp16 on the PE
(full-rate, FWL weight loads; fp32 PSUM accumulation).  Outputs are
stored fp16 (rounding ~5e-4 rel, tolerance is 2e-2) halving store
traffic; bias is added on the host during the gather.

Layouts are partition-major in DRAM so every DMA descriptor is a 2-8KB
contiguous run:
  xT[p, m, k, t] = x[m*128+t, k*128+p]   (lhsT tiles slice out [p, t])
  w [p, k, j]    = W[k*128+p, j]         (rhs slices [p, j-half])

Schedule: the single HWDGE ring drains sequentially and a chunk is
usable only at its 16th engine-completion, so chunk-ready time is
~(3.3us + 4-4.7us per cumulative MB ahead of it).  Front chunks are
split into 128KB halves (x m1-3 by k-half, W k1/k2 by n-half) and
ordered so every chunk's ready-time minus its consumption offset is
balanced (~the theoretical floor base + slope^2/7).  Nine cold warmup
matmuls on a zeroed tile keep the PE busy from the post-barrier point
to the DMA-gated stream start so HAM unthrottles to 2.4 GHz before the
real matmuls.  Phase 1 holds m0..3 in all 8 PSUM banks, k advancing in
DMA-arrival order (PE FIFO = emission order); its k7 pass runs m0
first so the copies free banks before phase 2 claims them.  Phase 2 is
n-major per m-tile so each half-bank closes 8 matmuls early and its
copy+store overlaps the other half; the last half is two 256-col
quarters so the tail after the final matmul is a [128,256] copy + 64KB
store.
"""

import numpy as np
import concourse.bacc as bacc
import concourse.mybir as mybir
import concourse.tile as tile
from concourse.bass_utils import run_bass_kernel_spmd

B, T, D = 4, 2048, 1024
RANK, FACTOR, SUB = 8, 64, 4
S = 4 * SUB  # 16
NCORES = 8
NTOK = B * T // NCORES  # 1024 token rows per core
P = 128
KT = D // P     # 8 contraction chunks
MT = NTOK // P  # 8 token tiles per core
NH = 512        # f_out half (one PSUM bank)

_cached_nc = None


def build_module():
    global _cached_nc
    if _cached_nc is not None:
        return _cached_nc
    nc = bacc.Bacc("TRN2", target_bir_lowering=False, debug=False)
    xT = nc.dram_tensor("xT", [P, MT, KT, P], mybir.dt.float16, kind="ExternalInput").ap()
    w = nc.dram_tensor("w", [P, KT, D], mybir.dt.float16, kind="ExternalInput").ap()
    out = nc.dram_tensor("out", [NTOK, D], mybir.dt.float16, kind="ExternalOutput").ap()

    with tile.TileContext(nc) as tc:
        with (
            tc.tile_pool(name="wp", bufs=1) as wp,
            tc.tile_pool(name="xp", bufs=1) as xp,
            tc.tile_pool(name="op", bufs=4) as op,
            tc.tile_pool(name="ps", bufs=4, space="PSUM") as ps,
        ):
            # PE HAM pre-warm: a couple of matmuls on a zeroed SBUF tile
            # start the activity window while the first loads are in
            # flight.  Tuned to end right as the first real operands
            # land — more would block the PE FIFO behind junk work.
            g = xp.tile([P, NH], mybir.dt.float16, tag="warm", name="g")
            nc.gpsimd.memset(g[:], 0.0)

            # Singles (256KB each, 2KB/partition contiguous) so every
            # ~0.95us another chunk unlocks matmuls; phase-2 x as pairs.
            KH = KT // 2
            xm = {0: xp.tile([P, 1, KT, P], mybir.dt.float16, tag="x0", name="xm0")}
            # m1..3 split into k-halves so the phase-1-critical first
            # half needs fewer bytes ahead of it on the ring.
            xmh = {
                (m, h): xp.tile([P, 1, KH, P], mybir.dt.float16, tag=f"x{m}{h}", name=f"xm{m}{h}")
                for m in (1, 2, 3) for h in (0, 1)
            }
            xm45 = xp.tile([P, 2, KT, P], mybir.dt.float16, tag="x45", name="xm45")
            xm67 = xp.tile([P, 2, KT, P], mybir.dt.float16, tag="x67", name="xm67")
            WSPLIT = (1, 2)  # W chunks loaded as n-halves
            wk = {
                k: wp.tile([P, 1, D], mybir.dt.float16, tag=f"w{k}", name=f"wk{k}")
                for k in range(KT) if k not in WSPLIT
            }
            wkh = {
                (k, n): wp.tile([P, 1, NH], mybir.dt.float16, tag=f"w{k}{n}", name=f"wk{k}{n}")
                for k in WSPLIT for n in (0, 1)
            }

            def xs(m, k):
                # lhsT [128 contraction rows, 128 tokens] for tile (m, k)
                if m == 0:
                    return xm[0][:, 0, k, :]
                if m < 4:
                    return xmh[(m, k // KH)][:, 0, k % KH, :]
                pair = xm45 if m < 6 else xm67
                return pair[:, m % 2, k, :]

            def wr(k, n):
                # rhs [128 contraction rows, 512 outs]
                if k in WSPLIT:
                    return wkh[(k, n)][:, 0, :]
                return wk[k][:, 0, n * NH:(n + 1) * NH]

            def wrc(k, c0, cw):
                # rhs [128 contraction rows, cw outs] at column offset c0
                if k in WSPLIT:
                    return wkh[(k, c0 // NH)][:, 0, c0 % NH:c0 % NH + cw]
                return wk[k][:, 0, c0:c0 + cw]

            # One HWDGE ring (sync), deadline order: the ring drains
            # sequentially, and the completion skew of the slowest SDMA
            # engine grows with cumulative bytes, so every chunk's
            # ready-time is ~(base + slope * bytes-ahead-of-it).  The
            # 128KB halves keep the early phase-1 dependencies low on
            # that line; the second halves ride after wk2.
            loads = [
                (wk[0][:], w[:, 0:1, :]),
                (xm[0][:], xT[:, 0:1]),
                (xmh[(1, 0)][:], xT[:, 1:2, 0:KH]),
                (wkh[(1, 0)][:], w[:, 1:2, 0:NH]),
                (xmh[(2, 0)][:], xT[:, 2:3, 0:KH]),
                (xmh[(3, 0)][:], xT[:, 3:4, 0:KH]),
                (wkh[(1, 1)][:], w[:, 1:2, NH:D]),
                (wkh[(2, 0)][:], w[:, 2:3, 0:NH]),
                (wkh[(2, 1)][:], w[:, 2:3, NH:D]),
                (wk[3][:], w[:, 3:4, :]),
                (wk[4][:], w[:, 4:5, :]),
                (xmh[(1, 1)][:], xT[:, 1:2, KH:KT]),
                (xmh[(2, 1)][:], xT[:, 2:3, KH:KT]),
                (xmh[(3, 1)][:], xT[:, 3:4, KH:KT]),
                (wk[5][:], w[:, 5:6, :]),
                (wk[6][:], w[:, 6:7, :]),
                (wk[7][:], w[:, 7:8, :]),
                (xm45[:], xT[:, 4:6]),
                (xm67[:], xT[:, 6:8]),
            ]
            for da, sa in loads:
                nc.sync.dma_start(da, sa)

            with nc.named_scope("mm"):
                pts = {
                    m: {
                        n: ps.tile([P, NH], mybir.dt.float32, tag=f"ps{n}", name=f"pt{m}_{n}")
                        for n in range(2)
                    }
                    for m in range(4)
                }
                # Cold warmups x 427ns bridge the PE from the post-
                # barrier point to the DMA-gated stream start (~5.3us in)
                # with no idle window, so HAM fires before the real
                # matmuls and every one of them runs at 2.4 GHz.
                for i in range(9):
                    nc.tensor.matmul(
                        pts[0][0][:], g[:, :P], g[:], start=True, stop=True
                    )

                # Phase 1: emission order tracks DMA arrival.
                def mm(m, k):
                    for n in range(2):
                        nc.tensor.matmul(
                            pts[m][n][:], xs(m, k), wr(k, n),
                            start=(k == 0), stop=(k == KT - 1),
                        )

                def mm1(m, k, n):
                    nc.tensor.matmul(
                        pts[m][n][:], xs(m, k), wr(k, n),
                        start=(k == 0), stop=(k == KT - 1),
                    )

                mm(0, 0); mm(1, 0)
                mm1(0, 1, 0); mm1(1, 1, 0)
                mm(2, 0); mm(3, 0)
                mm1(0, 1, 1); mm1(1, 1, 1)
                mm(2, 1); mm(3, 1)
                for m in range(4):
                    mm1(m, 2, 0)
                for m in range(4):
                    mm1(m, 2, 1)
                for k in range(3, KT):
                    for m in range(4):
                        mm(m, k)

                def emit_full(m, pt):
                    o = op.tile([P, D], mybir.dt.float16, tag="o", name="o")
                    for n in range(2):
                        nc.vector.tensor_copy(o[:, n * NH:(n + 1) * NH], pt[n][:])
                    nc.sync.dma_start(out[m * P:(m + 1) * P, :], o[:])

                for m in range(4):
                    emit_full(m, pts[m])

                # Phase 2: n-major per m so each half closes early.  The
                # very last half is split into two 256-col quarters so
                # the final copy+store tail after the last matmul is a
                # [128,256] copy + 64KB store instead of twice that.
                for m in range(4, MT):
                    last = m == MT - 1
                    pieces = [(0, NH), (NH, NH)] if not last else [
                        (0, NH), (NH, NH // 2), (NH + NH // 2, NH // 2)
                    ]
                    for n, (c0, cw) in enumerate(pieces):
                        pt = ps.tile([P, cw], mybir.dt.float32, tag=f"ps{min(n,1)}", name=f"p2_{m}_{n}")
                        for k in range(KT):
                            nc.tensor.matmul(
                                pt[:], xs(m, k), wrc(k, c0, cw),
                                start=(k == 0), stop=(k == KT - 1),
                            )
                        o = op.tile([P, cw], mybir.dt.float16, tag="oh", name="oh")
                        nc.vector.tensor_copy(o[:], pt[:])
                        nc.sync.dma_start(
                            out[m * P:(m + 1) * P, c0:c0 + cw], o[:]
                        )
    nc.compile()
    _cached_nc = nc
    return nc


def _construct_hamilton(A):
    # A: [rank, 4, sub, sub] -> [rank, 4*sub, 4*sub]
    r, i, j, k = A[:, 0], A[:, 1], A[:, 2], A[:, 3]
    return np.concatenate(
        [
            np.concatenate([r, -i, -j, -k], axis=2),
            np.concatenate([i, r, -k, j], axis=2),
            np.concatenate([j, k, r, -i], axis=2),
            np.concatenate([k, -j, i, r], axis=2),
        ],
        axis=1,
    )


def build_in_maps(x, A, factors_B):
    H = _construct_hamilton(np.asarray(A, dtype=np.float64))  # [r, k, s]
    Bf = np.asarray(factors_B, dtype=np.float64)  # [r, j, i]
    # W[(s,i),(k,j)] = sum_r H[r,k,s] * B[r,j,i]
    W = np.einsum("rks,rji->sikj", H, Bf).reshape(D, D).astype(np.float16)
    # w[p, k, j] = W[k*128+p, j] -> per-partition 16KB contiguous
    whost = np.ascontiguousarray(W.reshape(KT, P, D).transpose(1, 0, 2))

    x2 = np.asarray(x, dtype=np.float16).reshape(NCORES, NTOK, D)
    in_maps = []
    for c in range(NCORES):
        # xT[p, m, k, t] = x_core[m*128+t, k*128+p]
        xs = np.ascontiguousarray(
            x2[c].reshape(MT, P, KT, P).transpose(3, 0, 2, 1)
        )
        in_maps.append({"xT": xs, "w": whost})
    return in_maps


def kernel(x, A, factors_B, bias):
    nc = build_module()
    in_maps = build_in_maps(x, A, factors_B)
    br = run_bass_kernel_spmd(nc, in_maps, core_ids=list(range(NCORES)))
    out = np.concatenate([r["out"] for r in br.results], axis=0)
    out = out.astype(np.float32) + np.asarray(bias, dtype=np.float32)[None, :]
    return out.reshape(B, T, D).astype(np.float32)
